# revision 37
# baseline (speedup 1.0000x reference)
"""TRN2 Bass kernel for nn_KVGather: out[b,i,t] = kv[b, r_idx[b,i,t]] * r_weight[b,i,t].

Full shapes: r_idx/r_weight (32,49,4), kv (32,49,64,256) f32 -> out (32,49,4,64,256) f32.

Sharding: batch dim n=32 across 8 cores (4 batches/core), pure data parallel.

Per-core device kernel (memory-bound, bf16 I/O):
  - Gather+scale as a one-hot matmul with the *kv element-slice* stationary:
      psum[128 elems, 392 tiles] = kv2[98, 128].T @ W2[98, 392]
    where kv2 stacks the rows of a batch PAIR on 98 partitions (indices are
    batch-local) and W2[r, j] = r_weight[j] one-hot in r. 256 matmuls of
    392 moving columns — ~40% less PE time than tile-stationary chunks, and
    no dynamic APs or register loads.
  - PSUM f32 -> SBUF bf16 evictions [128, 392], alternating ACT/DVE
    (GPSIMD cannot access PSUM on TRN2).
  - Output DRAM layout is the blocked [pair, e, ec, j] transpose so each
    per-partition descriptor line spans KG=8 ec-blocks (6272 B — raises
    per-DMA-engine rate from ~18.5 to ~24.5 GB/s); host unpermutes for free.
  - All DMAs on the sync/HWDGE queue (SWDGE runs at half per-engine rate);
    kv pair loads are interleaved between output DMAs to avoid head-blocking.
"""

import os
import sys

sys.path.insert(0, "/opt/trn_rl_repo")

import numpy as np

N, P2, TOPK, HW_KV, C_KV = 32, 49, 4, 64, 256
NCORES = 8
NB = N // NCORES  # 4 batches per core
ROW = HW_KV * C_KV  # 16384 elems per kv row / output tile
HROW = ROW // 2  # 8192, kv row half held per (pair, half) SBUF tile
TPB = P2 * TOPK  # 196 output tiles per batch
TILES = NB * TPB  # 784 output tiles per core
NPAIR = 2  # batch pairs (0,1) and (2,3)
CP = 2 * P2  # 98 contraction rows per pair
MT = 2 * TPB  # 392 moving columns (= tiles of one pair)
EC = 128  # elems per matmul (stationary free dim)
NECH = HROW // EC  # 64 e-chunks per (pair, half)
KG = 8  # e-chunks per stage buffer / output DMA (6272 B descriptor lines)

# eviction engine split ACT:DVE proportional to modeled op rates
_N_EV = 128  # paired evictions (2 psum banks -> one strided copy)
_N_ACT = 70


def _ev_engine(i):
    return "A" if (i + 1) * _N_ACT // _N_EV - i * _N_ACT // _N_EV else "D"


_compiled = None


def _build():
    import concourse.bass as bass
    import concourse.tile as tile
    from concourse import bacc, mybir

    nc = bacc.Bacc("TRN2", target_bir_lowering=False, debug=False)

    f32 = mybir.dt.float32
    bf16 = mybir.dt.bfloat16
    COPY = mybir.ActivationFunctionType.Copy

    # kv reads run ~20 GB/s/engine vs ~25.6 for writes — an HBM read-path
    # property (line size and stride padding measured no effect)
    kv_d = nc.dram_tensor("kv", [CP, NPAIR, 2, HROW], bf16, kind="ExternalInput").ap()
    w_d = nc.dram_tensor("w", [CP, NPAIR, MT], bf16, kind="ExternalInput").ap()
    # [pair, e-partition, ec, j]: per-partition DMA lines span KG ec-blocks
    # (6272 B descriptors instead of 784 B)
    out_d = nc.dram_tensor("out", [NPAIR, EC, ROW // EC, MT], bf16, kind="ExternalOutput").ap()

    with tile.TileContext(nc) as tc:
        with (
            tc.tile_pool(name="res", bufs=1) as res_pool,
            tc.tile_pool(name="kvp", bufs=4) as kv_pool,
            tc.tile_pool(name="stage", bufs=6) as stage_pool,
            tc.tile_pool(name="psum", bufs=4, space=bass.MemorySpace.PSUM) as psum_pool,
        ):
            w_sb = res_pool.tile([CP, NPAIR, MT], bf16, tag="w")

            kvh = {}

            def load_kv(g, h):
                t = kv_pool.tile([CP, HROW], bf16, tag="kv")
                nc.sync.dma_start(t[:], kv_d[:, g, h, :])
                kvh[g, h] = t

            # kv(0,0) first so its transfer heads the DMA stream; the small
            # w transfer rides right behind it on the same queue
            load_kv(0, 0)
            nc.sync.dma_start(w_sb[:], w_d[:])

            ev_i = 0
            unit = 0
            for g in range(NPAIR):
                for h in range(2):
                    for kb in range(NECH // KG):
                        stage = stage_pool.tile([EC, KG * MT], bf16, tag="st")
                        st3 = stage[:].rearrange("e (kp j) -> e kp j", j=MT)
                        for kk in range(KG):
                            ecl = kb * KG + kk
                            if kk % 2 == 0:
                                # 2 psum banks; matmuls fill cols 0:392 of each
                                ps = psum_pool.tile([EC, 2, 512], f32, tag="ps")
                            nc.tensor.matmul(
                                ps[:, kk % 2, 0:MT],
                                kvh[g, h][:, ecl * EC : (ecl + 1) * EC],
                                w_sb[:, g, :],
                                start=True,
                                stop=True,
                            )
                            if kk % 2 == 0:
                                continue
                            # one strided-src eviction compacts both banks
                            dst = st3[:, kk - 1 : kk + 1, :]
                            src = ps[:, :, 0:MT]
                            if _ev_engine(ev_i) == "A":
                                nc.scalar.activation(dst, src, COPY)
                            else:
                                nc.vector.tensor_copy(dst, src)
                            ev_i += 1
                        ec0 = h * NECH + kb * KG
                        nc.sync.dma_start(
                            out_d[g, :, ec0 : ec0 + KG, :],
                            stage[:].rearrange("e (k j) -> e k j", j=MT),
                        )
                        unit += 1
                        if unit == 2:
                            load_kv(0, 1)
                        elif unit == 6:
                            load_kv(1, 0)
                        elif unit == 10:
                            load_kv(1, 1)

    nc.compile()
    return nc


def _get_compiled():
    global _compiled
    if _compiled is None:
        _compiled = _build()
    return _compiled


def _enable_trace_hook():
    """Register the axon NTFF profile hook (missing antenv.axon_hooks shim)."""
    import types

    try:
        import antenv.axon_hooks  # noqa: F401

        return
    except ImportError:
        pass
    try:
        import antenv

        mod = types.ModuleType("antenv.axon_hooks")
        holder = {}
        mod.set_axon_ntff_profile_hook = lambda h: holder.__setitem__("h", h)
        mod.get_axon_ntff_profile_hook = lambda: holder.get("h")
        antenv.axon_hooks = mod
        sys.modules["antenv.axon_hooks"] = mod
        if "/root/.axon_site" not in sys.path:
            sys.path.insert(0, "/root/.axon_site")
        from trn_agent_boot.trn_boot import _ntff_profile_via_ctypes

        mod.set_axon_ntff_profile_hook(
            _ntff_profile_via_ctypes("/opt/axon/libaxon_pjrt.so")
        )

        import concourse.bass_utils as bu

        orig = bu.upload_artifacts

        def _safe_upload(tmpdir):
            try:
                return orig(tmpdir)
            except Exception:
                return tmpdir
    except Exception as e:  # tracing is best-effort
        print(f"trace hook setup failed: {e}")


def kernel(r_idx, r_weight, kv):
    import ml_dtypes

    from concourse.bass_utils import run_bass_kernel_spmd

    bf16 = ml_dtypes.bfloat16

    r_idx = np.asarray(r_idx)
    r_weight = np.asarray(r_weight, dtype=np.float32)
    kv = np.asarray(kv, dtype=np.float32)
    assert r_idx.shape == (N, P2, TOPK) and kv.shape == (N, P2, HW_KV, C_KV)

    nc = _get_compiled()

    cols = np.arange(TPB)
    in_maps = []
    for c in range(NCORES):
        b0 = c * NB
        # kv2[bip*49 + row, g, h, e] = kv[b0 + 2g + bip, row, h*8192 + e]
        kv_c = kv[b0 : b0 + NB].reshape(NPAIR, 2, P2, 2, HROW)
        kvT2 = np.ascontiguousarray(kv_c.transpose(1, 2, 0, 3, 4)).reshape(
            CP, NPAIR, 2, HROW
        )
        idx4 = np.asarray(r_idx[b0 : b0 + NB], dtype=np.int64).reshape(NB, TPB)
        w4 = r_weight[b0 : b0 + NB].reshape(NB, TPB)
        W2 = np.zeros((CP, NPAIR, MT), dtype=np.float32)
        for g in range(NPAIR):
            for bip in range(2):
                b = 2 * g + bip
                W2[bip * P2 + idx4[b], g, bip * TPB + cols] = w4[b]
        in_maps.append({"kv": kvT2.astype(bf16), "w": W2.astype(bf16)})

    trace = bool(int(os.environ.get("KV_TRACE", "0")))
    if trace:
        _enable_trace_hook()
    res = run_bass_kernel_spmd(nc, in_maps, list(range(NCORES)), trace=trace)

    if trace:
        kernel.last_exec_time_ns = res.exec_time_ns
        kernel.last_trace = (
            res.instructions_and_trace[1] if res.instructions_and_trace else None
        )

    out = np.empty((N, P2, TOPK, HW_KV, C_KV), dtype=np.float32)
    for c in range(NCORES):
        b0 = c * NB
        a = np.asarray(res.results[c]["out"]).reshape(NPAIR, EC, ROW // EC, 2, TPB)
        a = a.transpose(0, 3, 4, 2, 1).reshape(NB, TPB, ROW)
        out[b0 : b0 + NB] = a.astype(np.float32).reshape(NB, P2, TOPK, HW_KV, C_KV)
    return out
